# revision 8
# baseline (speedup 1.0000x reference)
"""Trainium2 Bass kernel for nn_MixtureConfounderPrior.

Reference math (B,T,D=16,64,1024; K,CD,CF=32,128,128):
  cm  = 0.9*code_momentum + 0.1*code_embed
  mix = softmax(silu(h@mw_w1 + mw_b1) @ mw_w2 + mw_b2)
  mu_pre[t,k,c]  = (h@mu_w1[:D])[t,c] + (cm@mu_w1[D:])[k,c] + mu_b1[c]
  mus  = clip(tanh(LN(mu_pre)*g+b @ mu_w2 + mu_b2), -3, 3)
  lv   = clip((h@lv_w[:D])[t,c] + (cm@lv_w[D:])[k,c] + lv_b[c], LV_MIN, LV_MAX)

Key transformations used here:
  * mu_pre is rank-structured: A[t,c] + C[k,c].  LayerNorm stats collapse to
      mean[t,k] = mA[t]+mC[k],  var[t,k] = vA[t]+vC[k]+(2/CF)*(Ahat@Chat^T)[t,k]
  * the (t*k, CF)@(CF, CF) GEMM collapses to
      mus[t,k,f] = tanh(rstd[t,k]*(U[t,f]+V[k,f]) + bbias[f])
    with U = Ahat@(g*W2) on device and V = Chat@(g*W2) precomputed on host.
  * clip(tanh(x),-3,3) == tanh(x).
  * silu(x) = 0.5*x*(1+tanh(x/2)); the 0.5 is folded into mw_w2 so ACT only
    ever needs the exp/tanh table set (+ one sqrt for rstd).

Data parallel over batch: 8 cores x 2 batches (128 tokens each); weights and
code-derived constants replicated.  No collectives; host gathers the slices.
"""

import math

import numpy as np

import concourse.bass as bass
import concourse.mybir as mybir
import concourse.tile as tile
from concourse.bass_utils import run_bass_kernel_spmd

B, T, D = 16, 64, 1024
K, CD, CF = 32, 128, 128
MOM = 0.9
LN_EPS = 1e-5
LV_MIN, LV_MAX = math.log(0.1), math.log(2.0)
NCORES = 8
BPC = B // NCORES          # batches per core
TOK = BPC * T              # 128 tokens per core
DCH = D // 128             # 8 contraction chunks
KG = 4                     # codes per PSUM bank group
NG = K // KG               # 8 bank groups
F32 = mybir.dt.float32
AX = mybir.AluOpType


def _split_drain_waits(nc, max_waits=1):
    """walrus in this env rejects >1 sem wait per instruction and any sem
    wait on a Drain.  Hoist them onto NoOps placed just before."""
    for f in nc.m.functions:
        for bb in f.blocks:
            new_list = []
            for inst in bb.instructions:
                si = inst.sync_info
                if si is not None and si.on_wait:
                    is_drain = isinstance(inst, mybir.InstDrain)
                    keep = 0 if is_drain else max_waits
                    if len(si.on_wait) > keep:
                        waits = list(si.on_wait)
                        head = waits[: len(waits) - keep]
                        for i in range(0, len(head), 1):
                            new_list.append(
                                mybir.InstNoOp(
                                    name=f"{inst.name}-wsplit{i}",
                                    engine=inst.engine,
                                    sync_info=mybir.SyncInfo(
                                        on_wait=[head[i]], on_update=[]
                                    ),
                                )
                            )
                        si.on_wait = waits[len(waits) - keep :]
                new_list.append(inst)
            bb.instructions[:] = new_list


def build_bass(has_b1, has_b2, has_bb, split_waits=True):
    nc = bass.Bass("TRN2", num_devices=NCORES)

    def din(name, shape):
        return nc.dram_tensor(name, shape, F32, kind="ExternalInput")

    h_d = din("h_loc", (TOK, D))
    w1s_d = din("w1s", (128, DCH, 256))
    wmu_d = din("wmu", (128, DCH, CF))
    wlv_d = din("wlv", (128, DCH, CF))
    w2m_d = din("w2m", (128, 2, K))
    i4_d = din("i4", (128, KG * CF))
    w2g4_d = din("w2g4", (128, KG * CF))
    vflat_d = din("vflat", (1, K * CF))
    clv_d = din("clvflat", (1, K * CF))
    chT_d = din("chT", (CF, K))
    vcs_d = din("vcs", (1, K))
    ones_d = din("ones_row", (1, 128))
    b1h_d = din("b1h", (128, 2)) if has_b1 else None
    b1f_d = din("b1f", (128, 2)) if has_b1 else None
    b2_d = din("b2row", (1, K)) if has_b2 else None
    bb_d = din("bbrep", (128, CF)) if has_bb else None

    mixw_d = nc.dram_tensor("mixw", (TOK, K), F32, kind="ExternalOutput")
    mus_d = nc.dram_tensor("mus", (TOK, K, CF), F32, kind="ExternalOutput")
    lv_d = nc.dram_tensor("lv", (TOK, K, CF), F32, kind="ExternalOutput")

    from contextlib import ExitStack

    with tile.TileContext(nc) as tc, ExitStack() as ctx:
        cons = ctx.enter_context(tc.tile_pool(name="cons", bufs=1))
        psA = ctx.enter_context(tc.tile_pool(name="psA", bufs=2, space="PSUM"))
        psB = ctx.enter_context(tc.tile_pool(name="psB", bufs=2, space="PSUM"))
        psS = ctx.enter_context(tc.tile_pool(name="psS", bufs=1, space="PSUM"))
        psP = ctx.enter_context(tc.tile_pool(name="psP", bufs=2, space="PSUM"))
        psL = ctx.enter_context(tc.tile_pool(name="psL", bufs=1, space="PSUM"))
        stg = ctx.enter_context(tc.tile_pool(name="stg", bufs=3))

        # ---- loads -------------------------------------------------------
        h_sb = cons.tile([TOK, D], F32)
        nc.sync.dma_start(h_sb, h_d.ap())
        w1s = cons.tile([128, DCH, 256], F32)
        nc.sync.dma_start(w1s, w1s_d.ap())
        wmu = cons.tile([128, DCH, CF], F32)
        nc.sync.dma_start(wmu, wmu_d.ap())
        wlv = cons.tile([128, DCH, CF], F32)
        nc.sync.dma_start(wlv, wlv_d.ap())
        w2m = cons.tile([128, 2, K], F32)
        nc.sync.dma_start(w2m, w2m_d.ap())
        i4 = cons.tile([128, KG * CF], F32)
        nc.sync.dma_start(i4, i4_d.ap())
        w2g4 = cons.tile([128, KG * CF], F32)
        nc.sync.dma_start(w2g4, w2g4_d.ap())
        vflat = cons.tile([1, K * CF], F32)
        nc.sync.dma_start(vflat, vflat_d.ap())
        clv = cons.tile([1, K * CF], F32)
        nc.sync.dma_start(clv, clv_d.ap())
        chT = cons.tile([CF, K], F32)
        nc.sync.dma_start(chT, chT_d.ap())
        vcs = cons.tile([1, K], F32)
        nc.sync.dma_start(vcs, vcs_d.ap())
        ones = cons.tile([1, 128], F32)
        nc.sync.dma_start(ones, ones_d.ap())
        if has_b1:
            b1h = cons.tile([128, 2], F32)
            nc.sync.dma_start(b1h, b1h_d.ap())
            b1f = cons.tile([128, 2], F32)
            nc.sync.dma_start(b1f, b1f_d.ap())
        if has_b2:
            b2r = cons.tile([1, K], F32)
            nc.sync.dma_start(b2r, b2_d.ap())
        if has_bb:
            bbr = cons.tile([128, CF], F32)
            nc.sync.dma_start(bbr, bb_d.ap())

        # ---- X^T: 8 PE transposes of h ----------------------------------
        ident = i4[:, 0:128]
        xT = cons.tile([128, DCH, TOK], F32)
        for kc in range(DCH):
            pt = psA.tile([128, 128], F32, tag="pt")
            nc.tensor.transpose(pt, h_sb[:, kc * 128 : (kc + 1) * 128], ident)
            nc.scalar.copy(xT[:, kc, :], pt)

        # ---- mu stats path (Sqrt first on ACT to minimize table loads) ---
        p_amu = psB.tile([TOK, CF], F32, tag="mm")
        for kc in range(DCH):
            nc.tensor.matmul(p_amu, lhsT=xT[:, kc, :], rhs=wmu[:, kc, :],
                             start=(kc == 0), stop=(kc == DCH - 1))
        stats = cons.tile([TOK, 6], F32)
        nc.vector.bn_stats(stats, p_amu)
        mv = cons.tile([TOK, 2], F32)
        nc.vector.bn_aggr(mv, stats)
        vae = cons.tile([TOK, 1], F32)
        nc.vector.tensor_scalar(vae, mv[:, 1:2], LN_EPS, None, AX.add)
        ahat = cons.tile([TOK, CF], F32)
        nc.vector.tensor_scalar(ahat, p_amu, mv[:, 0:1], None, AX.subtract)
        p_at = psA.tile([128, 128], F32, tag="pt")
        nc.tensor.transpose(p_at, ahat, ident)
        aT = cons.tile([CF, TOK], F32)
        nc.scalar.copy(aT, p_at)

        p_s = psS.tile([TOK, K], F32, tag="ps")
        nc.tensor.matmul(p_s, lhsT=aT, rhs=chT, start=True, stop=False)
        nc.tensor.matmul(p_s, lhsT=ones, rhs=vcs, start=False, stop=True)
        sd = cons.tile([TOK, K], F32)
        nc.scalar.activation(sd, p_s, mybir.ActivationFunctionType.Sqrt,
                             bias=vae, scale=2.0 / CF)
        rstd = cons.tile([TOK, K], F32)
        nc.vector.reciprocal(rstd, sd)

        # ---- A_lv^T ------------------------------------------------------
        p_alvt = psB.tile([CF, TOK], F32, tag="mm")
        for kc in range(DCH):
            nc.tensor.matmul(p_alvt, lhsT=wlv[:, kc, :], rhs=xT[:, kc, :],
                             start=(kc == 0), stop=(kc == DCH - 1))
        alvT = cons.tile([CF, TOK], F32)
        nc.scalar.copy(alvT, p_alvt)

        # ---- mix path ----------------------------------------------------
        y1s = cons.tile([128, 2, TOK], F32)
        for j in range(2):
            p_y1 = psB.tile([128, TOK], F32, tag="mm")
            for kc in range(DCH):
                nc.tensor.matmul(p_y1, lhsT=w1s[:, kc, j * 128 : (j + 1) * 128],
                                 rhs=xT[:, kc, :],
                                 start=(kc == 0), stop=(kc == DCH - 1))
            th = cons.tile([128, TOK], F32, tag=f"th{j}")
            if has_b1:
                nc.scalar.activation(th, p_y1, mybir.ActivationFunctionType.Tanh,
                                     bias=b1h[:, j : j + 1], scale=0.5)
                y1 = cons.tile([128, TOK], F32, tag=f"y1{j}")
                nc.vector.tensor_scalar(y1, p_y1, b1f[:, j : j + 1], None, AX.add)
            else:
                nc.scalar.activation(th, p_y1, mybir.ActivationFunctionType.Tanh,
                                     scale=0.5)
                y1 = cons.tile([128, TOK], F32, tag=f"y1{j}")
                nc.vector.tensor_copy(y1, p_y1)
            tmp = cons.tile([128, TOK], F32, tag=f"tmp{j}")
            nc.vector.tensor_tensor(tmp, y1, th, AX.mult)
            nc.vector.tensor_tensor(y1s[:, j, :], tmp, y1, AX.add)

        p_z = psS.tile([TOK, K], F32, tag="ps")
        nc.tensor.matmul(p_z, lhsT=y1s[:, 0, :], rhs=w2m[:, 0, :],
                         start=True, stop=False)
        nc.tensor.matmul(p_z, lhsT=y1s[:, 1, :], rhs=w2m[:, 1, :],
                         start=False, stop=not has_b2)
        if has_b2:
            nc.tensor.matmul(p_z, lhsT=ones, rhs=b2r, start=False, stop=True)
        mx = cons.tile([TOK, 1], F32)
        nc.vector.reduce_max(mx, p_z, axis=mybir.AxisListType.X)
        nmx = cons.tile([TOK, 1], F32)
        nc.vector.tensor_scalar(nmx, mx, -1.0, None, AX.mult)
        ez = cons.tile([TOK, K], F32)
        esum = cons.tile([TOK, 1], F32)
        nc.scalar.activation(ez, p_z, mybir.ActivationFunctionType.Exp,
                             bias=nmx, accum_out=esum)
        rsum = cons.tile([TOK, 1], F32)
        nc.vector.reciprocal(rsum, esum)
        mixw = cons.tile([TOK, K], F32)
        nc.vector.tensor_scalar(mixw, ez, rsum, None, AX.mult)
        nc.sync.dma_start(mixw_d.ap(), mixw)

        # ---- big outputs: mus and logvars per 4-code group --------------
        for g in range(NG):
            sl = slice(g * KG * CF, (g + 1) * KG * CF)
            p_p = psP.tile([TOK, KG, CF], F32, tag="pp")
            nc.tensor.matmul(p_p, lhsT=aT, rhs=w2g4, start=True, stop=False)
            nc.tensor.matmul(p_p, lhsT=ones, rhs=vflat[:, sl],
                             start=False, stop=True)
            st = stg.tile([TOK, KG, CF], F32, tag="st")
            nc.vector.tensor_tensor(
                st, p_p,
                rstd[:, g * KG : (g + 1) * KG, None].to_broadcast((TOK, KG, CF)),
                AX.mult)
            if has_bb:
                nc.vector.tensor_tensor(
                    st, st, bbr[:, None, :].to_broadcast((TOK, KG, CF)), AX.add)
            mus_sb = stg.tile([TOK, KG, CF], F32, tag="mu")
            nc.scalar.activation(mus_sb, st, mybir.ActivationFunctionType.Tanh)
            nc.sync.dma_start(mus_d.ap()[:, g * KG : (g + 1) * KG, :], mus_sb)

            p_l = psL.tile([TOK, KG, CF], F32, tag="pl")
            nc.tensor.matmul(p_l, lhsT=alvT, rhs=i4, start=True, stop=False)
            nc.tensor.matmul(p_l, lhsT=ones, rhs=clv[:, sl],
                             start=False, stop=True)
            lv_sb = stg.tile([TOK, KG, CF], F32, tag="lv")
            nc.vector.tensor_scalar(lv_sb, p_l, LV_MAX, LV_MIN, AX.min, AX.max)
            nc.sync.dma_start(lv_d.ap()[:, g * KG : (g + 1) * KG, :], lv_sb)

    if split_waits:
        _split_drain_waits(nc)
    return nc


def prepare(inputs):
    """Host-side preprocessing -> (in_maps, flags). All heavy per-token work
    stays on device; only (K,CD)-sized code/weight constants are folded."""
    f64 = {k: np.asarray(v, np.float64) for k, v in inputs.items()}
    h = np.ascontiguousarray(np.asarray(inputs["h"], np.float32))

    cm = MOM * f64["code_momentum"] + (1.0 - MOM) * f64["code_embed"]
    Cmu = cm @ f64["mu_w1"][D:] + f64["mu_b1"]          # (K, CF)
    mC = Cmu.mean(-1, keepdims=True)
    Chat = Cmu - mC
    vC = (Chat**2).mean(-1)                              # (K,)
    W2g = f64["ln_g"][:, None] * f64["mu_w2"]            # (CF, CF)
    V = Chat @ W2g                                       # (K, CF)
    bbias = f64["ln_b"] @ f64["mu_w2"] + f64["mu_b2"]    # (CF,)
    Clv = cm @ f64["lv_w"][D:] + f64["lv_b"]             # (K, CF)

    c = lambda a: np.ascontiguousarray(np.asarray(a, np.float32))
    w1s = c(f64["mw_w1"].reshape(DCH, 128, 256).transpose(1, 0, 2))
    wmu = c(f64["mu_w1"][:D].reshape(DCH, 128, CF).transpose(1, 0, 2))
    wlv = c(f64["lv_w"][:D].reshape(DCH, 128, CF).transpose(1, 0, 2))
    w2m = c((0.5 * f64["mw_w2"]).reshape(2, 128, K).transpose(1, 0, 2))
    i4 = c(np.tile(np.eye(128), (1, KG)))
    w2g4 = c(np.tile(W2g, (1, KG)))
    vflat = c(V.reshape(1, K * CF))
    clvflat = c(Clv.reshape(1, K * CF))
    chT = c(Chat.T)
    vcs = c(((CF / 2.0) * vC).reshape(1, K))
    ones_row = np.ones((1, 128), np.float32)

    has_b1 = bool(np.any(f64["mw_b1"]))
    has_b2 = bool(np.any(f64["mw_b2"]))
    has_bb = bool(np.any(bbias))

    common = dict(w1s=w1s, wmu=wmu, wlv=wlv, w2m=w2m, i4=i4, w2g4=w2g4,
                  vflat=vflat, clvflat=clvflat, chT=chT, vcs=vcs,
                  ones_row=ones_row)
    if has_b1:
        common["b1h"] = c(0.5 * f64["mw_b1"].reshape(2, 128).T)
        common["b1f"] = c(f64["mw_b1"].reshape(2, 128).T)
    if has_b2:
        common["b2row"] = c(f64["mw_b2"].reshape(1, K))
    if has_bb:
        common["bbrep"] = c(np.tile(bbias.reshape(1, CF), (128, 1)))

    in_maps = []
    for i in range(NCORES):
        m = dict(common)
        m["h_loc"] = np.ascontiguousarray(
            h[i * BPC : (i + 1) * BPC].reshape(TOK, D))
        in_maps.append(m)
    return in_maps, (has_b1, has_b2, has_bb)


_CACHE = {}


def run(inputs, **spmd_kwargs):
    in_maps, flags = prepare(inputs)
    if flags not in _CACHE:
        _CACHE[flags] = build_bass(*flags)
    nc = _CACHE[flags]
    res = run_bass_kernel_spmd(nc, in_maps, core_ids=list(range(NCORES)),
                               **spmd_kwargs)
    mix = np.empty((B, T, K), np.float32)
    mus = np.empty((B, T, K, CF), np.float32)
    lv = np.empty((B, T, K, CF), np.float32)
    for i, r in enumerate(res.results):
        sl = slice(i * BPC, (i + 1) * BPC)
        mix[sl] = r["mixw"].reshape(BPC, T, K)
        mus[sl] = r["mus"].reshape(BPC, T, K, CF)
        lv[sl] = r["lv"].reshape(BPC, T, K, CF)
    return (mix, mus, lv), res


def kernel(**inputs):
    out, _ = run(inputs)
    return out


# revision 12
# speedup vs baseline: 1.7985x; 1.7985x over previous
"""Trainium2 Bass kernel for nn_MixtureConfounderPrior.

Reference math (B,T,D=16,64,1024; K,CD,CF=32,128,128):
  cm  = 0.9*code_momentum + 0.1*code_embed
  mix = softmax(silu(h@mw_w1 + mw_b1) @ mw_w2 + mw_b2)
  mu_pre[t,k,c]  = (h@mu_w1[:D])[t,c] + (cm@mu_w1[D:])[k,c] + mu_b1[c]
  mus  = clip(tanh(LN(mu_pre)*g+b @ mu_w2 + mu_b2), -3, 3)
  lv   = clip((h@lv_w[:D])[t,c] + (cm@lv_w[D:])[k,c] + lv_b[c], LV_MIN, LV_MAX)

Key transformations:
  * mu_pre is rank-structured: A[t,c] + C[k,c].  LayerNorm stats collapse to
      mean[t,k] = mA[t]+mC[k],  var[t,k] = vA[t]+vC[k]+(2/CF)*(Ahat@Chat^T)[t,k]
  * the (t*k, CF)@(CF, CF) GEMM collapses to
      mus[t,k,f] = tanh(rstd[t,k]*(U[t,f]+V[k,f]) + bbias[f])
    with U = Ahat@(g*W2) on device and V = Chat@(g*W2) precomputed on host.
    The k-broadcasts are built in PSUM: U replicated via a 4x-tiled rhs,
    V/C_lv added via ones-row rank-1 accumulate matmuls.
  * clip(tanh(x),-3,3) == tanh(x); tanh(rstd*P) fused on ACT via per-partition
    scale = rstd[:,k].
  * silu(x) = 0.5*x*(1+tanh(x/2)); the 0.5 folds into mw_w2 so ACT needs only
    the exp/tanh table set (+ one Sqrt for rstd, ordered first).
  * matmuls with free dim >= 256 run in float32r (1 cyc/row vs 4 for fp32,
    ~1e-4 rel err).  PE transposes stay fp32 (fp32r transpose is broken on HW).

Data parallel over batch: 8 cores x 2 batches (128 tokens each); weights and
code-derived constants replicated.  No collectives; host gathers the slices.
"""

import math
from contextlib import ExitStack

import numpy as np

import concourse.bass as bass
import concourse.mybir as mybir
import concourse.tile as tile
from concourse.bass_utils import run_bass_kernel_spmd
from concourse.tile import add_dep_helper

B, T, D = 16, 64, 1024
K, CD, CF = 32, 128, 128
MOM = 0.9
LN_EPS = 1e-5
LV_MIN, LV_MAX = math.log(0.1), math.log(2.0)
NCORES = 8
BPC = B // NCORES          # batches per core
TOK = BPC * T              # 128 tokens per core
DCH = D // 128             # 8 contraction chunks
KG = 4                     # codes per PSUM bank group
NG = K // KG               # 8 bank groups
F32 = mybir.dt.float32
F32R = mybir.dt.float32r
AX = mybir.AluOpType
AF = mybir.ActivationFunctionType


def _split_drain_waits(nc, max_waits=1):
    """walrus in this env rejects >1 sem wait per instruction and any sem
    wait on a Drain.  Hoist them onto NoOps placed just before."""
    for f in nc.m.functions:
        for bb in f.blocks:
            new_list = []
            for inst in bb.instructions:
                si = inst.sync_info
                if si is not None and si.on_wait:
                    keep = 0 if isinstance(inst, mybir.InstDrain) else max_waits
                    if len(si.on_wait) > keep:
                        waits = list(si.on_wait)
                        head = waits[: len(waits) - keep]
                        for i, w in enumerate(head):
                            new_list.append(
                                mybir.InstNoOp(
                                    name=f"{inst.name}-wsplit{i}",
                                    engine=inst.engine,
                                    sync_info=mybir.SyncInfo(
                                        on_wait=[w], on_update=[]
                                    ),
                                )
                            )
                        si.on_wait = waits[len(waits) - keep :]
                new_list.append(inst)
            bb.instructions[:] = new_list


# smalls layout: [vflat 4096 | clvflat 4096 | ones 128 | vcs 32]
SM_V, SM_C, SM_1, SM_S = 0, K * CF, 2 * K * CF, 2 * K * CF + 128
SM_LEN = SM_S + K


def build_bass(has_b1, has_b2, has_bb, split_waits=True):
    nc = bass.Bass("TRN2", num_devices=NCORES)

    def din(name, shape, dt=F32R):
        return nc.dram_tensor(name, shape, dt, kind="ExternalInput")

    h_d = din("h_loc", (TOK, D), F32)
    ident_d = din("ident", (128, 128), F32)
    w1s_d = din("w1s", (128, DCH, 256))
    wml_d = din("wml", (128, DCH, 256))
    w2m_d = din("w2m", (128, 2, K), F32)
    i4_d = din("i4", (128, KG * CF))
    w2g4_d = din("w2g4", (128, KG * CF))
    sm_d = din("smalls", (1, SM_LEN))
    chT_d = din("chT", (CF, K), F32)
    b1_d = din("b1row", (1, 256), F32) if has_b1 else None
    b2_d = din("b2row", (1, K), F32) if has_b2 else None
    bb_d = din("bbrep", (128, CF), F32) if has_bb else None

    mixw_d = nc.dram_tensor("mixw", (TOK, K), F32, kind="ExternalOutput")
    mus_d = nc.dram_tensor("mus", (TOK, K, CF), F32, kind="ExternalOutput")
    lv_d = nc.dram_tensor("lv", (TOK, K, CF), F32, kind="ExternalOutput")

    with tile.TileContext(nc) as tc, ExitStack() as ctx:
        cons = ctx.enter_context(tc.tile_pool(name="cons", bufs=1))
        stg = ctx.enter_context(tc.tile_pool(name="stg", bufs=3))

        # ---- loads -------------------------------------------------------
        def load(name, shape, d, dt=F32R):
            t = cons.tile(shape, dt, tag=name, name=name)
            nc.sync.dma_start(t, d.ap())
            return t

        h_sb = load("h", [TOK, D], h_d, F32)
        ident = load("ident", [128, 128], ident_d, F32)
        wml = load("wml", [128, DCH, 256], wml_d)
        chT = load("chT", [CF, K], chT_d, F32)
        sm = load("sm", [1, SM_LEN], sm_d)
        w1s = load("w1s", [128, DCH, 256], w1s_d)
        w2m = load("w2m", [128, 2, K], w2m_d, F32)
        w2g4 = load("w2g4", [128, KG * CF], w2g4_d)
        i4 = load("i4", [128, KG * CF], i4_d)
        if has_b1:
            b1w = cons.tile([TOK, 256], F32)
            nc.sync.dma_start(
                b1w, bass.AP(tensor=b1_d, offset=0, ap=[[0, TOK], [1, 256]]))
        if has_b2:
            b2r = load("b2r", [1, K], b2_d, F32)
        if has_bb:
            bbr = load("bbr", [128, CF], bb_d, F32)

        vflat = sm[:, SM_V : SM_V + K * CF]
        clv = sm[:, SM_C : SM_C + K * CF]
        ones = sm[:, SM_1 : SM_1 + 128]
        vcs = sm[:, SM_S : SM_S + K].bitcast(F32)

        with tc.tile_pool(name="pt", bufs=2, space="PSUM") as ptp, \
             tc.tile_pool(name="mm", bufs=2, space="PSUM") as mmp, \
             tc.tile_pool(name="pss", bufs=1, space="PSUM") as pss:

            # ---- X^T: 8 PE transposes of h (fp32) ------------------------
            xT = cons.tile([128, DCH, TOK], F32R)
            for kc in range(DCH):
                pt = ptp.tile([128, 128], F32, tag="pt")
                nc.tensor.transpose(pt, h_sb[:, kc * 128 : (kc + 1) * 128],
                                    ident)
                nc.vector.tensor_copy(xT[:, kc, :], pt)

            # ---- A_mu | A_lv in one N=256 fp32r group --------------------
            p_ml = mmp.tile([TOK, 256], F32, tag="mm")
            for kc in range(DCH):
                nc.tensor.matmul(p_ml, lhsT=xT[:, kc, :], rhs=wml[:, kc, :],
                                 start=(kc == 0), stop=(kc == DCH - 1))
            stats = cons.tile([TOK, 6], F32)
            nc.vector.bn_stats(stats, p_ml[:, 0:CF])
            mv = cons.tile([TOK, 2], F32)
            nc.vector.bn_aggr(mv, stats)
            vae = cons.tile([TOK, 1], F32)
            nc.vector.tensor_scalar(vae, mv[:, 1:2], LN_EPS, None, AX.add)
            ahat = cons.tile([TOK, CF], F32)
            nc.vector.tensor_scalar(ahat, p_ml[:, 0:CF], mv[:, 0:1], None,
                                    AX.subtract)
            alv = cons.tile([TOK, CF], F32)
            nc.vector.tensor_copy(alv, p_ml[:, CF:256])

            p_at = ptp.tile([128, 128], F32, tag="pt")
            nc.tensor.transpose(p_at, ahat, ident)
            aT = cons.tile([CF, TOK], F32R)
            nc.scalar.copy(aT, p_at)
            p_lt = ptp.tile([128, 128], F32, tag="pt")
            nc.tensor.transpose(p_lt, alv, ident)
            alvT = cons.tile([CF, TOK], F32R)
            nc.scalar.copy(alvT, p_lt)

            # ---- rstd: var = vA + vC + (2/CF)*Ahat@ChatT (fp32, N=32) ----
            p_s = pss.tile([TOK, K], F32, tag="ps")
            nc.tensor.matmul(p_s, lhsT=aT.bitcast(F32), rhs=chT,
                             start=True, stop=False)
            nc.tensor.matmul(p_s, lhsT=ones.bitcast(F32), rhs=vcs,
                             start=False, stop=True)
            sd = cons.tile([TOK, K], F32)
            sd_i = nc.scalar.activation(sd, p_s, AF.Sqrt, bias=vae,
                                        scale=2.0 / CF)
            rstd = cons.tile([TOK, K], F32)
            nc.vector.reciprocal(rstd, sd)

            # ---- mix path: Y1 = h@w1 (N=256 fp32r), silu via tanh --------
            p_y1 = mmp.tile([TOK, 256], F32, tag="mm")
            for kc in range(DCH):
                nc.tensor.matmul(p_y1, lhsT=xT[:, kc, :], rhs=w1s[:, kc, :],
                                 start=(kc == 0), stop=(kc == DCH - 1))
            th = cons.tile([TOK, 256], F32)
            y1 = cons.tile([TOK, 256], F32)
            if has_b1:
                nc.vector.tensor_tensor(y1, p_y1, b1w, AX.add)
                th_i = nc.scalar.activation(th, y1, AF.Tanh, scale=0.5)
            else:
                nc.vector.tensor_copy(y1, p_y1)
                th_i = nc.scalar.activation(th, p_y1, AF.Tanh, scale=0.5)
            # keep ACT table order: Sqrt before first Tanh
            add_dep_helper(th_i.ins, sd_i.ins, sync=False,
                           reason="ACT table-set order (sqrt first)")
            tmp = cons.tile([TOK, 256], F32)
            nc.vector.tensor_tensor(tmp, y1, th, AX.mult)
            y1s = cons.tile([TOK, 256], F32)
            nc.vector.tensor_tensor(y1s, tmp, y1, AX.add)

            y1sT = cons.tile([128, 2, TOK], F32)
            for j in range(2):
                p_yt = ptp.tile([128, 128], F32, tag="pt")
                nc.tensor.transpose(p_yt, y1s[:, j * 128 : (j + 1) * 128],
                                    ident)
                nc.scalar.copy(y1sT[:, j, :], p_yt)

            p_z = pss.tile([TOK, K], F32, tag="ps")
            nc.tensor.matmul(p_z, lhsT=y1sT[:, 0, :], rhs=w2m[:, 0, :],
                             start=True, stop=False)
            nc.tensor.matmul(p_z, lhsT=y1sT[:, 1, :], rhs=w2m[:, 1, :],
                             start=False, stop=not has_b2)
            if has_b2:
                nc.tensor.matmul(p_z, lhsT=ones.bitcast(F32), rhs=b2r,
                                 start=False, stop=True)
            mx = cons.tile([TOK, 1], F32)
            nc.vector.reduce_max(mx, p_z, axis=mybir.AxisListType.X)
            nmx = cons.tile([TOK, 1], F32)
            nc.vector.tensor_scalar(nmx, mx, -1.0, None, AX.mult)
            ez = cons.tile([TOK, K], F32)
            esum = cons.tile([TOK, 1], F32)
            nc.scalar.activation(ez, p_z, AF.Exp, bias=nmx, accum_out=esum)
            rsum = cons.tile([TOK, 1], F32)
            nc.vector.reciprocal(rsum, esum)
            mixw = cons.tile([TOK, K], F32)
            nc.vector.tensor_scalar(mixw, ez, rsum, None, AX.mult)
            nc.sync.dma_start(mixw_d.ap(), mixw)

        # ---- big outputs: 2 halves x 4 banks, weights kept stationary ----
        with tc.tile_pool(name="pP", bufs=4, space="PSUM") as psP, \
             tc.tile_pool(name="pL", bufs=4, space="PSUM") as psL:
            for half in range(2):
                gs = [half * 4 + q for q in range(4)]
                Pb = {}
                for g in gs:
                    Pb[g] = psP.tile([TOK, KG, CF], F32, tag="pp", name=f"pp{g}")
                    nc.tensor.matmul(Pb[g], lhsT=aT, rhs=w2g4,
                                     start=True, stop=False)
                for g in gs:
                    sl = slice(SM_V + g * KG * CF, SM_V + (g + 1) * KG * CF)
                    nc.tensor.matmul(Pb[g], lhsT=ones, rhs=sm[:, sl],
                                     start=False, stop=True)
                for g in gs:
                    src = Pb[g]
                    if has_bb:
                        st = stg.tile([TOK, KG, CF], F32, tag="st")
                        nc.vector.tensor_tensor(
                            st, src,
                            bbr[:, None, :].to_broadcast((TOK, KG, CF)),
                            AX.add)
                        src = st
                    mus_sb = stg.tile([TOK, KG, CF], F32, tag="mu")
                    for kk in range(KG):
                        k = g * KG + kk
                        nc.scalar.activation(mus_sb[:, kk, :], src[:, kk, :],
                                             AF.Tanh,
                                             scale=rstd[:, k : k + 1])
                    nc.sync.dma_start(mus_d.ap()[:, g * KG : (g + 1) * KG, :],
                                      mus_sb)
                Lb = {}
                for g in gs:
                    Lb[g] = psL.tile([TOK, KG, CF], F32, tag="pl", name=f"pl{g}")
                    nc.tensor.matmul(Lb[g], lhsT=alvT, rhs=i4,
                                     start=True, stop=False)
                for g in gs:
                    sl = slice(SM_C + g * KG * CF, SM_C + (g + 1) * KG * CF)
                    nc.tensor.matmul(Lb[g], lhsT=ones, rhs=sm[:, sl],
                                     start=False, stop=True)
                for g in gs:
                    lv_sb = stg.tile([TOK, KG, CF], F32, tag="lv")
                    nc.vector.tensor_scalar(lv_sb, Lb[g], LV_MAX, LV_MIN,
                                            AX.min, AX.max)
                    nc.sync.dma_start(lv_d.ap()[:, g * KG : (g + 1) * KG, :],
                                      lv_sb)

    if split_waits:
        _split_drain_waits(nc)
    return nc


def prepare(inputs):
    """Host-side preprocessing -> (in_maps, flags). All heavy per-token work
    stays on device; only (K,CD)-sized code/weight constants are folded."""
    f64 = {k: np.asarray(v, np.float64) for k, v in inputs.items()}
    h = np.ascontiguousarray(np.asarray(inputs["h"], np.float32))

    cm = MOM * f64["code_momentum"] + (1.0 - MOM) * f64["code_embed"]
    Cmu = cm @ f64["mu_w1"][D:] + f64["mu_b1"]          # (K, CF)
    mC = Cmu.mean(-1, keepdims=True)
    Chat = Cmu - mC
    vC = (Chat**2).mean(-1)                              # (K,)
    W2g = f64["ln_g"][:, None] * f64["mu_w2"]            # (CF, CF)
    V = Chat @ W2g                                       # (K, CF)
    bbias = f64["ln_b"] @ f64["mu_w2"] + f64["mu_b2"]    # (CF,)
    Clv = cm @ f64["lv_w"][D:] + f64["lv_b"]             # (K, CF)

    c = lambda a: np.ascontiguousarray(np.asarray(a, np.float32))
    w1s = c(f64["mw_w1"].reshape(DCH, 128, 256).transpose(1, 0, 2))
    wmu = f64["mu_w1"][:D].reshape(DCH, 128, CF).transpose(1, 0, 2)
    wlv = f64["lv_w"][:D].reshape(DCH, 128, CF).transpose(1, 0, 2)
    wml = c(np.concatenate([wmu, wlv], axis=2))          # (128, DCH, 256)
    w2m = c((0.5 * f64["mw_w2"]).reshape(2, 128, K).transpose(1, 0, 2))
    i4 = c(np.tile(np.eye(128), (1, KG)))
    w2g4 = c(np.tile(W2g, (1, KG)))
    ident = c(np.eye(128))
    smalls = np.zeros((1, SM_LEN), np.float32)
    smalls[0, SM_V : SM_V + K * CF] = V.reshape(-1)
    smalls[0, SM_C : SM_C + K * CF] = Clv.reshape(-1)
    smalls[0, SM_1 : SM_1 + 128] = 1.0
    smalls[0, SM_S : SM_S + K] = (CF / 2.0) * vC
    chT = c(Chat.T)

    has_b1 = bool(np.any(f64["mw_b1"]))
    has_b2 = bool(np.any(f64["mw_b2"]))
    has_bb = bool(np.any(bbias))

    common = dict(w1s=w1s, wml=wml, w2m=w2m, i4=i4, w2g4=w2g4,
                  smalls=smalls, chT=chT, ident=ident)
    if has_b1:
        common["b1row"] = c(f64["mw_b1"].reshape(1, 256))
    if has_b2:
        common["b2row"] = c(f64["mw_b2"].reshape(1, K))
    if has_bb:
        common["bbrep"] = c(np.tile(bbias.reshape(1, CF), (128, 1)))

    in_maps = []
    for i in range(NCORES):
        m = dict(common)
        m["h_loc"] = np.ascontiguousarray(
            h[i * BPC : (i + 1) * BPC].reshape(TOK, D))
        in_maps.append(m)
    return in_maps, (has_b1, has_b2, has_bb)


_CACHE = {}


def run(inputs, **spmd_kwargs):
    in_maps, flags = prepare(inputs)
    if flags not in _CACHE:
        _CACHE[flags] = build_bass(*flags)
    nc = _CACHE[flags]
    res = run_bass_kernel_spmd(nc, in_maps, core_ids=list(range(NCORES)),
                               **spmd_kwargs)
    mix = np.empty((B, T, K), np.float32)
    mus = np.empty((B, T, K, CF), np.float32)
    lv = np.empty((B, T, K, CF), np.float32)
    for i, r in enumerate(res.results):
        sl = slice(i * BPC, (i + 1) * BPC)
        mix[sl] = r["mixw"].reshape(BPC, T, K)
        mus[sl] = r["mus"].reshape(BPC, T, K, CF)
        lv[sl] = r["lv"].reshape(BPC, T, K, CF)
    return (mix, mus, lv), res


def kernel(**inputs):
    out, _ = run(inputs)
    return out


# revision 14
# speedup vs baseline: 1.8290x; 1.0169x over previous
"""Trainium2 Bass kernel for nn_MixtureConfounderPrior.

Reference math (B,T,D=16,64,1024; K,CD,CF=32,128,128):
  cm  = 0.9*code_momentum + 0.1*code_embed
  mix = softmax(silu(h@mw_w1 + mw_b1) @ mw_w2 + mw_b2)
  mu_pre[t,k,c]  = (h@mu_w1[:D])[t,c] + (cm@mu_w1[D:])[k,c] + mu_b1[c]
  mus  = clip(tanh(LN(mu_pre)*g+b @ mu_w2 + mu_b2), -3, 3)
  lv   = clip((h@lv_w[:D])[t,c] + (cm@lv_w[D:])[k,c] + lv_b[c], LV_MIN, LV_MAX)

Key transformations:
  * mu_pre is rank-structured: A[t,c] + C[k,c].  LayerNorm stats collapse to
      mean[t,k] = mA[t]+mC[k],  var[t,k] = vA[t]+vC[k]+(2/CF)*(Ahat@Chat^T)[t,k]
  * the (t*k, CF)@(CF, CF) GEMM collapses to
      mus[t,k,f] = tanh(rstd[t,k]*(U[t,f]+V[k,f]) + bbias[f])
    with U = Ahat@(g*W2) on device and V = Chat@(g*W2) precomputed on host.
    The k-broadcasts are built in PSUM: U replicated via a 4x-tiled rhs,
    V/C_lv added via ones-row rank-1 accumulate matmuls.
  * clip(tanh(x),-3,3) == tanh(x); tanh(rstd*P) fused on ACT via per-partition
    scale = rstd[:,k].
  * silu(x) = 0.5*x*(1+tanh(x/2)); the 0.5 folds into mw_w2 so ACT needs only
    the exp/tanh table set (+ one Sqrt for rstd, ordered first).
  * matmuls with free dim >= 256 run in float32r (1 cyc/row vs 4 for fp32,
    ~1e-4 rel err).  PE transposes stay fp32 (fp32r transpose is broken on HW).

Data parallel over batch: 8 cores x 2 batches (128 tokens each); weights and
code-derived constants replicated.  No collectives; host gathers the slices.
"""

import math
from contextlib import ExitStack

import numpy as np

import concourse.bass as bass
import concourse.mybir as mybir
import concourse.tile as tile
from concourse.bass_utils import run_bass_kernel_spmd
from concourse.tile import add_dep_helper

B, T, D = 16, 64, 1024
K, CD, CF = 32, 128, 128
MOM = 0.9
LN_EPS = 1e-5
LV_MIN, LV_MAX = math.log(0.1), math.log(2.0)
NCORES = 8
BPC = B // NCORES          # batches per core
TOK = BPC * T              # 128 tokens per core
DCH = D // 128             # 8 contraction chunks
KG = 4                     # codes per PSUM bank group
NG = K // KG               # 8 bank groups
F32 = mybir.dt.float32
F32R = mybir.dt.float32r
AX = mybir.AluOpType
AF = mybir.ActivationFunctionType


def _split_drain_waits(nc, max_waits=1):
    """walrus in this env rejects >1 sem wait per instruction and any sem
    wait on a Drain.  Hoist them onto NoOps placed just before."""
    for f in nc.m.functions:
        for bb in f.blocks:
            new_list = []
            for inst in bb.instructions:
                si = inst.sync_info
                if si is not None and si.on_wait:
                    keep = 0 if isinstance(inst, mybir.InstDrain) else max_waits
                    if len(si.on_wait) > keep:
                        waits = list(si.on_wait)
                        head = waits[: len(waits) - keep]
                        for i, w in enumerate(head):
                            new_list.append(
                                mybir.InstNoOp(
                                    name=f"{inst.name}-wsplit{i}",
                                    engine=inst.engine,
                                    sync_info=mybir.SyncInfo(
                                        on_wait=[w], on_update=[]
                                    ),
                                )
                            )
                        si.on_wait = waits[len(waits) - keep :]
                new_list.append(inst)
            bb.instructions[:] = new_list


# pack32 column layout (f32 cols): h | ident | wml | w1s | w2g4 | w2m | chT
# | ones32(row0) | vcs(row0)
H0, ID, WML, W1S, W2G, I4C, W2M, CHT, ON32, VCS, P32_LEN = (
    0, 1024, 1152, 3200, 5248, 5760, 6272, 6336, 6368, 6496, 6528)
# smalls16 (bf16, partition 0): vflat | clvflat | ones16
SM_V, SM_C, SM_1, SM_LEN = 0, K * CF, 2 * K * CF, 2 * K * CF + 128
BF16 = mybir.dt.bfloat16


def build_bass(has_b1, has_b2, has_bb, split_waits=True):
    nc = bass.Bass("TRN2", num_devices=NCORES)

    def din(name, shape, dt=F32R):
        return nc.dram_tensor(name, shape, dt, kind="ExternalInput")

    p32_d = din("pack32", (128, P32_LEN))
    sm_d = din("smalls16", (1, SM_LEN), BF16)
    b1_d = din("b1row", (1, 256), F32) if has_b1 else None
    b2_d = din("b2row", (1, K), F32) if has_b2 else None
    bb_d = din("bbrep", (128, CF), F32) if has_bb else None

    mixw_d = nc.dram_tensor("mixw", (TOK, K), F32, kind="ExternalOutput")
    mus_d = nc.dram_tensor("mus", (TOK, K, CF), F32, kind="ExternalOutput")
    lv_d = nc.dram_tensor("lv", (TOK, K, CF), F32, kind="ExternalOutput")

    with tile.TileContext(nc) as tc, ExitStack() as ctx:
        cons = ctx.enter_context(tc.tile_pool(name="cons", bufs=1))
        stg = ctx.enter_context(tc.tile_pool(name="stg", bufs=3))

        # ---- loads: 3 chunked triggers for pack32 + 2 small packs --------
        p32 = cons.tile([128, P32_LEN], F32R, tag="p32", name="p32")
        p32a = p32_d.ap()
        nc.sync.dma_start(p32[:, H0:WML], p32a[:, H0:WML])      # h + ident
        nc.sync.dma_start(p32[:, WML:W1S], p32a[:, WML:W1S])    # wml
        nc.sync.dma_start(p32[:, W1S:P32_LEN], p32a[:, W1S:P32_LEN])
        sm = cons.tile([1, SM_LEN], BF16, tag="sm", name="sm")
        nc.sync.dma_start(sm, sm_d.ap())

        h_sb = p32[:, H0:ID].bitcast(F32)
        ident = p32[:, ID:WML].bitcast(F32)
        wml = p32[:, WML:W1S].rearrange("p (a b) -> p a b", b=256)
        w1s = p32[:, W1S:W2G].rearrange("p (a b) -> p a b", b=256)
        w2g4 = p32[:, W2G:I4C]
        i4r = p32[:, I4C:W2M]
        w2m = p32[:, W2M:CHT].bitcast(F32).rearrange("p (a b) -> p a b", b=K)
        chT = p32[:, CHT:ON32].bitcast(F32)
        ones32 = p32[0:1, ON32:VCS].bitcast(F32)
        vcs = p32[0:1, VCS:P32_LEN].bitcast(F32)
        if has_b1:
            b1w = cons.tile([TOK, 256], F32)
            nc.sync.dma_start(
                b1w, bass.AP(tensor=b1_d, offset=0, ap=[[0, TOK], [1, 256]]))
        if has_b2:
            b2r = load("b2r", [1, K], b2_d, F32)
        if has_bb:
            bbr = load("bbr", [128, CF], bb_d, F32)

        vflat = sm[:, SM_V : SM_V + K * CF]
        clv = sm[:, SM_C : SM_C + K * CF]
        ones16 = sm[:, SM_1 : SM_1 + 128]

        with tc.tile_pool(name="pt", bufs=2, space="PSUM") as ptp, \
             tc.tile_pool(name="mm", bufs=2, space="PSUM") as mmp, \
             tc.tile_pool(name="pss", bufs=1, space="PSUM") as pss:

            # ---- X^T: 8 PE transposes of h (fp32) ------------------------
            xT = cons.tile([128, DCH, TOK], F32R)
            for kc in range(DCH):
                pt = ptp.tile([128, 128], F32, tag="pt")
                nc.tensor.transpose(pt, h_sb[:, kc * 128 : (kc + 1) * 128],
                                    ident)
                nc.vector.tensor_copy(xT[:, kc, :], pt)

            # ---- A_mu | A_lv in one N=256 fp32r group --------------------
            p_ml = mmp.tile([TOK, 256], F32, tag="mm")
            for kc in range(DCH):
                nc.tensor.matmul(p_ml, lhsT=xT[:, kc, :], rhs=wml[:, kc, :],
                                 start=(kc == 0), stop=(kc == DCH - 1))
            stats = cons.tile([TOK, 6], F32)
            nc.vector.bn_stats(stats, p_ml[:, 0:CF])
            mv = cons.tile([TOK, 2], F32)
            nc.vector.bn_aggr(mv, stats)
            vae = cons.tile([TOK, 1], F32)
            nc.vector.tensor_scalar(vae, mv[:, 1:2], LN_EPS, None, AX.add)
            ahat = cons.tile([TOK, CF], F32)
            nc.vector.tensor_scalar(ahat, p_ml[:, 0:CF], mv[:, 0:1], None,
                                    AX.subtract)
            alv = cons.tile([TOK, CF], F32)
            nc.vector.tensor_copy(alv, p_ml[:, CF:256])

            p_at = ptp.tile([128, 128], F32, tag="pt")
            nc.tensor.transpose(p_at, ahat, ident)
            aT = cons.tile([CF, TOK], F32R)
            nc.scalar.copy(aT, p_at)
            p_lt = ptp.tile([128, 128], F32, tag="pt")
            nc.tensor.transpose(p_lt, alv, ident)
            alvT = cons.tile([CF, TOK], F32R)
            nc.scalar.copy(alvT, p_lt)

            # ---- rstd: var = vA + vC + (2/CF)*Ahat@ChatT (fp32, N=32) ----
            p_s = pss.tile([TOK, K], F32, tag="ps")
            nc.tensor.matmul(p_s, lhsT=aT.bitcast(F32), rhs=chT,
                             start=True, stop=False)
            nc.tensor.matmul(p_s, lhsT=ones32, rhs=vcs,
                             start=False, stop=True)
            sd = cons.tile([TOK, K], F32)
            sd_i = nc.scalar.activation(sd, p_s, AF.Sqrt, bias=vae,
                                        scale=2.0 / CF)
            rstd = cons.tile([TOK, K], F32)
            nc.vector.reciprocal(rstd, sd)

            # ---- mix path: Y1 = h@w1 (N=256 fp32r), silu via tanh --------
            p_y1 = mmp.tile([TOK, 256], F32, tag="mm")
            for kc in range(DCH):
                nc.tensor.matmul(p_y1, lhsT=xT[:, kc, :], rhs=w1s[:, kc, :],
                                 start=(kc == 0), stop=(kc == DCH - 1))
            th = cons.tile([TOK, 256], F32)
            y1 = cons.tile([TOK, 256], F32)
            if has_b1:
                nc.vector.tensor_tensor(y1, p_y1, b1w, AX.add)
                th_i = nc.scalar.activation(th, y1, AF.Tanh, scale=0.5)
            else:
                nc.vector.tensor_copy(y1, p_y1)
                th_i = nc.scalar.activation(th, p_y1, AF.Tanh, scale=0.5)
            # keep ACT table order: Sqrt before first Tanh
            add_dep_helper(th_i.ins, sd_i.ins, sync=False,
                           reason="ACT table-set order (sqrt first)")
            tmp = cons.tile([TOK, 256], F32)
            nc.vector.tensor_tensor(tmp, y1, th, AX.mult)
            y1s = cons.tile([TOK, 256], F32)
            nc.vector.tensor_tensor(y1s, tmp, y1, AX.add)

            y1sT = cons.tile([128, 2, TOK], F32)
            for j in range(2):
                p_yt = ptp.tile([128, 128], F32, tag="pt")
                nc.tensor.transpose(p_yt, y1s[:, j * 128 : (j + 1) * 128],
                                    ident)
                nc.scalar.copy(y1sT[:, j, :], p_yt)

            p_z = pss.tile([TOK, K], F32, tag="ps")
            nc.tensor.matmul(p_z, lhsT=y1sT[:, 0, :], rhs=w2m[:, 0, :],
                             start=True, stop=False)
            nc.tensor.matmul(p_z, lhsT=y1sT[:, 1, :], rhs=w2m[:, 1, :],
                             start=False, stop=not has_b2)
            if has_b2:
                nc.tensor.matmul(p_z, lhsT=ones32, rhs=b2r,
                                 start=False, stop=True)
            mx = cons.tile([TOK, 1], F32)
            nc.vector.reduce_max(mx, p_z, axis=mybir.AxisListType.X)
            nmx = cons.tile([TOK, 1], F32)
            nc.vector.tensor_scalar(nmx, mx, -1.0, None, AX.mult)
            ez = cons.tile([TOK, K], F32)
            esum = cons.tile([TOK, 1], F32)
            nc.scalar.activation(ez, p_z, AF.Exp, bias=nmx, accum_out=esum)
            rsum = cons.tile([TOK, 1], F32)
            nc.vector.reciprocal(rsum, esum)
            mixw = cons.tile([TOK, K], F32)
            nc.vector.tensor_scalar(mixw, ez, rsum, None, AX.mult)
            nc.sync.dma_start(mixw_d.ap(), mixw)

        # ---- big outputs: 2 halves x 4 banks, weights kept stationary ----
        with tc.tile_pool(name="pP", bufs=4, space="PSUM") as psP, \
             tc.tile_pool(name="pL", bufs=4, space="PSUM") as psL:
            for half in range(2):
                gs = [half * 4 + q for q in range(4)]
                Pb = {}
                for g in gs:
                    Pb[g] = psP.tile([TOK, KG, CF], F32, tag="pp", name=f"pp{g}")
                    nc.tensor.matmul(Pb[g], lhsT=aT, rhs=w2g4,
                                     start=True, stop=False)
                for g in gs:
                    sl = slice(SM_V + g * KG * CF, SM_V + (g + 1) * KG * CF)
                    nc.tensor.matmul(Pb[g], lhsT=ones16, rhs=sm[:, sl],
                                     start=False, stop=True)
                for g in gs:
                    src = Pb[g]
                    if has_bb:
                        st = stg.tile([TOK, KG, CF], F32, tag="st")
                        nc.vector.tensor_tensor(
                            st, src,
                            bbr[:, None, :].to_broadcast((TOK, KG, CF)),
                            AX.add)
                        src = st
                    mus_sb = stg.tile([TOK, KG, CF], F32, tag="mu")
                    for kk in range(KG):
                        k = g * KG + kk
                        nc.scalar.activation(mus_sb[:, kk, :], src[:, kk, :],
                                             AF.Tanh,
                                             scale=rstd[:, k : k + 1])
                    nc.sync.dma_start(mus_d.ap()[:, g * KG : (g + 1) * KG, :],
                                      mus_sb)
                Lb = {}
                for g in gs:
                    Lb[g] = psL.tile([TOK, KG, CF], F32, tag="pl", name=f"pl{g}")
                    nc.tensor.matmul(Lb[g], lhsT=alvT, rhs=i4r,
                                     start=True, stop=False)
                for g in gs:
                    sl = slice(SM_C + g * KG * CF, SM_C + (g + 1) * KG * CF)
                    nc.tensor.matmul(Lb[g], lhsT=ones16, rhs=sm[:, sl],
                                     start=False, stop=True)
                for g in gs:
                    lv_sb = stg.tile([TOK, KG, CF], F32, tag="lv")
                    nc.vector.tensor_scalar(lv_sb, Lb[g], LV_MAX, LV_MIN,
                                            AX.min, AX.max)
                    nc.sync.dma_start(lv_d.ap()[:, g * KG : (g + 1) * KG, :],
                                      lv_sb)

    if split_waits:
        _split_drain_waits(nc)
    return nc


def prepare(inputs):
    """Host-side preprocessing -> (in_maps, flags). All heavy per-token work
    stays on device; only (K,CD)-sized code/weight constants are folded."""
    f64 = {k: np.asarray(v, np.float64) for k, v in inputs.items()}
    h = np.ascontiguousarray(np.asarray(inputs["h"], np.float32))

    cm = MOM * f64["code_momentum"] + (1.0 - MOM) * f64["code_embed"]
    Cmu = cm @ f64["mu_w1"][D:] + f64["mu_b1"]          # (K, CF)
    mC = Cmu.mean(-1, keepdims=True)
    Chat = Cmu - mC
    vC = (Chat**2).mean(-1)                              # (K,)
    W2g = f64["ln_g"][:, None] * f64["mu_w2"]            # (CF, CF)
    V = Chat @ W2g                                       # (K, CF)
    bbias = f64["ln_b"] @ f64["mu_w2"] + f64["mu_b2"]    # (CF,)
    Clv = cm @ f64["lv_w"][D:] + f64["lv_b"]             # (K, CF)

    import ml_dtypes
    c = lambda a: np.ascontiguousarray(np.asarray(a, np.float32))
    w1s = c(f64["mw_w1"].reshape(DCH, 128, 256).transpose(1, 0, 2))
    wmu = f64["mu_w1"][:D].reshape(DCH, 128, CF).transpose(1, 0, 2)
    wlv = f64["lv_w"][:D].reshape(DCH, 128, CF).transpose(1, 0, 2)
    wml = c(np.concatenate([wmu, wlv], axis=2))          # (128, DCH, 256)
    w2m = c((0.5 * f64["mw_w2"]).reshape(2, 128, K).transpose(1, 0, 2))
    w2g4 = c(np.tile(W2g, (1, KG)))
    chT = c(Chat.T)
    pack32 = np.zeros((128, P32_LEN), np.float32)
    # h filled per core below
    pack32[:, ID:WML] = np.eye(128, dtype=np.float32)
    pack32[:, WML:W1S] = wml.reshape(128, -1)
    pack32[:, W1S:W2G] = w1s.reshape(128, -1)
    pack32[:, W2G:I4C] = w2g4
    pack32[:, I4C:W2M] = np.tile(np.eye(128), (1, KG))
    pack32[:, W2M:CHT] = w2m.reshape(128, -1)
    pack32[:, CHT:ON32] = chT
    pack32[0, ON32:VCS] = 1.0
    pack32[0, VCS:P32_LEN] = (CF / 2.0) * vC
    smalls = np.zeros((1, SM_LEN), ml_dtypes.bfloat16)
    smalls[0, SM_V : SM_V + K * CF] = V.reshape(-1).astype(ml_dtypes.bfloat16)
    smalls[0, SM_C : SM_C + K * CF] = Clv.reshape(-1).astype(
        ml_dtypes.bfloat16)
    smalls[0, SM_1 : SM_1 + 128] = 1.0

    has_b1 = bool(np.any(f64["mw_b1"]))
    has_b2 = bool(np.any(f64["mw_b2"]))
    has_bb = bool(np.any(bbias))

    common = dict(smalls16=smalls)
    if has_b1:
        common["b1row"] = c(f64["mw_b1"].reshape(1, 256))
    if has_b2:
        common["b2row"] = c(f64["mw_b2"].reshape(1, K))
    if has_bb:
        common["bbrep"] = c(np.tile(bbias.reshape(1, CF), (128, 1)))

    in_maps = []
    for i in range(NCORES):
        m = dict(common)
        p = pack32.copy()
        p[:, H0:ID] = h[i * BPC : (i + 1) * BPC].reshape(TOK, D)
        m["pack32"] = p
        in_maps.append(m)
    return in_maps, (has_b1, has_b2, has_bb)


_CACHE = {}


def run(inputs, **spmd_kwargs):
    in_maps, flags = prepare(inputs)
    if flags not in _CACHE:
        _CACHE[flags] = build_bass(*flags)
    nc = _CACHE[flags]
    res = run_bass_kernel_spmd(nc, in_maps, core_ids=list(range(NCORES)),
                               **spmd_kwargs)
    mix = np.empty((B, T, K), np.float32)
    mus = np.empty((B, T, K, CF), np.float32)
    lv = np.empty((B, T, K, CF), np.float32)
    for i, r in enumerate(res.results):
        sl = slice(i * BPC, (i + 1) * BPC)
        mix[sl] = r["mixw"].reshape(BPC, T, K)
        mus[sl] = r["mus"].reshape(BPC, T, K, CF)
        lv[sl] = r["lv"].reshape(BPC, T, K, CF)
    return (mix, mus, lv), res


def kernel(**inputs):
    out, _ = run(inputs)
    return out


# revision 15
# speedup vs baseline: 2.0204x; 1.1046x over previous
"""Trainium2 Bass kernel for nn_MixtureConfounderPrior.

Reference math (B,T,D=16,64,1024; K,CD,CF=32,128,128):
  cm  = 0.9*code_momentum + 0.1*code_embed
  mix = softmax(silu(h@mw_w1 + mw_b1) @ mw_w2 + mw_b2)
  mu_pre[t,k,c]  = (h@mu_w1[:D])[t,c] + (cm@mu_w1[D:])[k,c] + mu_b1[c]
  mus  = clip(tanh(LN(mu_pre)*g+b @ mu_w2 + mu_b2), -3, 3)
  lv   = clip((h@lv_w[:D])[t,c] + (cm@lv_w[D:])[k,c] + lv_b[c], LV_MIN, LV_MAX)

Key transformations:
  * mu_pre is rank-structured: A[t,c] + C[k,c].  LayerNorm stats collapse to
      mean[t,k] = mA[t]+mC[k],  var[t,k] = vA[t]+vC[k]+(2/CF)*(Ahat@Chat^T)[t,k]
  * the (t*k, CF)@(CF, CF) GEMM collapses to
      mus[t,k,f] = tanh(rstd[t,k]*(U[t,f]+V[k,f]) + bbias[f])
    with U = Ahat@(g*W2) on device and V = Chat@(g*W2) precomputed on host.
    The k-broadcasts are built in PSUM: U replicated via a 4x-tiled rhs,
    V/C_lv added via ones-row rank-1 accumulate matmuls.
  * clip(tanh(x),-3,3) == tanh(x); tanh(rstd*P) fused on ACT via per-partition
    scale = rstd[:,k].
  * silu(x) = 0.5*x*(1+tanh(x/2)); the 0.5 folds into mw_w2 so ACT needs only
    the exp/tanh table set (+ one Sqrt for rstd, ordered first).
  * matmuls with free dim >= 256 run in float32r (1 cyc/row vs 4 for fp32,
    ~1e-4 rel err).  PE transposes stay fp32 (fp32r transpose is broken on HW).

Data parallel over batch: 8 cores x 2 batches (128 tokens each); weights and
code-derived constants replicated.  No collectives; host gathers the slices.
"""

import math
from contextlib import ExitStack

import numpy as np

import concourse.bass as bass
import concourse.mybir as mybir
import concourse.tile as tile
from concourse.bass_utils import run_bass_kernel_spmd
from concourse.tile import add_dep_helper

B, T, D = 16, 64, 1024
K, CD, CF = 32, 128, 128
MOM = 0.9
LN_EPS = 1e-5
LV_MIN, LV_MAX = math.log(0.1), math.log(2.0)
NCORES = 8
BPC = B // NCORES          # batches per core
TOK = BPC * T              # 128 tokens per core
DCH = D // 128             # 8 contraction chunks
KG = 4                     # codes per PSUM bank group
NG = K // KG               # 8 bank groups
F32 = mybir.dt.float32
F32R = mybir.dt.float32r
AX = mybir.AluOpType
AF = mybir.ActivationFunctionType


def _split_drain_waits(nc, max_waits=1):
    """walrus in this env rejects >1 sem wait per instruction and any sem
    wait on a Drain.  Hoist them onto NoOps placed just before."""
    for f in nc.m.functions:
        for bb in f.blocks:
            new_list = []
            for inst in bb.instructions:
                si = inst.sync_info
                if si is not None and si.on_wait:
                    keep = 0 if isinstance(inst, mybir.InstDrain) else max_waits
                    if len(si.on_wait) > keep:
                        waits = list(si.on_wait)
                        head = waits[: len(waits) - keep]
                        for i, w in enumerate(head):
                            new_list.append(
                                mybir.InstNoOp(
                                    name=f"{inst.name}-wsplit{i}",
                                    engine=inst.engine,
                                    sync_info=mybir.SyncInfo(
                                        on_wait=[w], on_update=[]
                                    ),
                                )
                            )
                        si.on_wait = waits[len(waits) - keep :]
                new_list.append(inst)
            bb.instructions[:] = new_list


# pack32 column layout (f32 cols): h | ident | wml | w1s | w2g4 | w2m | chT
# | ones32(row0) | vcs(row0)
H0, ID, WML, W1S, W2G, I4C, W2M, CHT, ON32, VCS, P32_LEN = (
    0, 1024, 1152, 3200, 5248, 5760, 6272, 6336, 6368, 6496, 6528)
# smalls16 (bf16, partition 0): vflat | clvflat | ones16
SM_V, SM_C, SM_1, SM_LEN = 0, K * CF, 2 * K * CF, 2 * K * CF + 128
BF16 = mybir.dt.bfloat16


def build_bass(has_b1, has_b2, has_bb, split_waits=True):
    nc = bass.Bass("TRN2", num_devices=NCORES)

    def din(name, shape, dt=F32R):
        return nc.dram_tensor(name, shape, dt, kind="ExternalInput")

    p32_d = din("pack32", (128, P32_LEN))
    sm_d = din("smalls16", (1, SM_LEN), BF16)
    b1_d = din("b1row", (1, 256), F32) if has_b1 else None
    b2_d = din("b2row", (1, K), F32) if has_b2 else None
    bb_d = din("bbrep", (128, CF), F32) if has_bb else None

    mixw_d = nc.dram_tensor("mixw", (TOK, K), F32, kind="ExternalOutput")
    mus_d = nc.dram_tensor("mus", (TOK, K, CF), F32, kind="ExternalOutput")
    lv_d = nc.dram_tensor("lv", (TOK, K, CF), F32, kind="ExternalOutput")

    with tile.TileContext(nc) as tc, ExitStack() as ctx:
        cons = ctx.enter_context(tc.tile_pool(name="cons", bufs=1))
        stg = ctx.enter_context(tc.tile_pool(name="stg", bufs=3))

        # ---- loads: 3 chunked triggers for pack32 + 2 small packs --------
        p32 = cons.tile([128, P32_LEN], F32R, tag="p32", name="p32")
        p32a = p32_d.ap()
        nc.gpsimd.dma_start(p32[:, H0:WML], p32a[:, H0:WML])    # h + ident
        nc.gpsimd.dma_start(p32[:, WML:W1S], p32a[:, WML:W1S])  # wml
        nc.gpsimd.dma_start(p32[:, W1S:P32_LEN], p32a[:, W1S:P32_LEN])
        sm = cons.tile([1, SM_LEN], BF16, tag="sm", name="sm")
        nc.gpsimd.dma_start(sm, sm_d.ap())

        h_sb = p32[:, H0:ID].bitcast(F32)
        ident = p32[:, ID:WML].bitcast(F32)
        wml = p32[:, WML:W1S].rearrange("p (a b) -> p a b", b=256)
        w1s = p32[:, W1S:W2G].rearrange("p (a b) -> p a b", b=256)
        w2g4 = p32[:, W2G:I4C]
        i4r = p32[:, I4C:W2M]
        w2m = p32[:, W2M:CHT].bitcast(F32).rearrange("p (a b) -> p a b", b=K)
        chT = p32[:, CHT:ON32].bitcast(F32)
        ones32 = p32[0:1, ON32:VCS].bitcast(F32)
        vcs = p32[0:1, VCS:P32_LEN].bitcast(F32)
        if has_b1:
            b1w = cons.tile([TOK, 256], F32)
            nc.sync.dma_start(
                b1w, bass.AP(tensor=b1_d, offset=0, ap=[[0, TOK], [1, 256]]))
        if has_b2:
            b2r = load("b2r", [1, K], b2_d, F32)
        if has_bb:
            bbr = load("bbr", [128, CF], bb_d, F32)

        vflat = sm[:, SM_V : SM_V + K * CF]
        clv = sm[:, SM_C : SM_C + K * CF]
        ones16 = sm[:, SM_1 : SM_1 + 128]

        with tc.tile_pool(name="pt", bufs=4, space="PSUM") as ptp, \
             tc.tile_pool(name="mm", bufs=2, space="PSUM") as mmp, \
             tc.tile_pool(name="pss", bufs=1, space="PSUM") as pss:

            # ---- X^T: 8 PE transposes of h (fp32) ------------------------
            xT = cons.tile([128, DCH, TOK], F32R)
            for kc in range(DCH):
                pt = ptp.tile([128, 128], F32, tag="pt")
                nc.tensor.transpose(pt, h_sb[:, kc * 128 : (kc + 1) * 128],
                                    ident)
                nc.vector.tensor_copy(xT[:, kc, :], pt)

            # ---- A_mu | A_lv in one N=256 fp32r group --------------------
            p_ml = mmp.tile([TOK, 256], F32, tag="mm")
            for kc in range(DCH):
                nc.tensor.matmul(p_ml, lhsT=xT[:, kc, :], rhs=wml[:, kc, :],
                                 start=(kc == 0), stop=(kc == DCH - 1))
            stats = cons.tile([TOK, 6], F32)
            nc.vector.bn_stats(stats, p_ml[:, 0:CF])
            mv = cons.tile([TOK, 2], F32)
            nc.vector.bn_aggr(mv, stats)
            vae = cons.tile([TOK, 1], F32)
            nc.vector.tensor_scalar(vae, mv[:, 1:2], LN_EPS, None, AX.add)
            ahat = cons.tile([TOK, CF], F32)
            nc.vector.tensor_scalar(ahat, p_ml[:, 0:CF], mv[:, 0:1], None,
                                    AX.subtract)
            alv = cons.tile([TOK, CF], F32)
            nc.vector.tensor_copy(alv, p_ml[:, CF:256])

            p_at = ptp.tile([128, 128], F32, tag="pt")
            nc.tensor.transpose(p_at, ahat, ident)
            aT = cons.tile([CF, TOK], F32R)
            nc.scalar.copy(aT, p_at)
            p_lt = ptp.tile([128, 128], F32, tag="pt")
            nc.tensor.transpose(p_lt, alv, ident)
            alvT = cons.tile([CF, TOK], F32R)
            nc.scalar.copy(alvT, p_lt)

            # ---- rstd: var = vA + vC + (2/CF)*Ahat@ChatT (fp32, N=32) ----
            p_s = pss.tile([TOK, K], F32, tag="ps")
            nc.tensor.matmul(p_s, lhsT=aT.bitcast(F32), rhs=chT,
                             start=True, stop=False)
            nc.tensor.matmul(p_s, lhsT=ones32, rhs=vcs,
                             start=False, stop=True)
            sd = cons.tile([TOK, K], F32)
            sd_i = nc.scalar.activation(sd, p_s, AF.Sqrt, bias=vae,
                                        scale=2.0 / CF)
            rstd = cons.tile([TOK, K], F32)
            nc.vector.reciprocal(rstd, sd)

            # ---- mix path: Y1 = h@w1 (N=256 fp32r), silu via tanh --------
            p_y1 = mmp.tile([TOK, 256], F32, tag="mm")
            for kc in range(DCH):
                nc.tensor.matmul(p_y1, lhsT=xT[:, kc, :], rhs=w1s[:, kc, :],
                                 start=(kc == 0), stop=(kc == DCH - 1))
            th = cons.tile([TOK, 256], F32)
            y1 = cons.tile([TOK, 256], F32)
            if has_b1:
                nc.vector.tensor_tensor(y1, p_y1, b1w, AX.add)
                th_i = nc.scalar.activation(th, y1, AF.Tanh, scale=0.5)
            else:
                nc.vector.tensor_copy(y1, p_y1)
                th_i = nc.scalar.activation(th, p_y1, AF.Tanh, scale=0.5)
            # keep ACT table order: Sqrt before first Tanh
            add_dep_helper(th_i.ins, sd_i.ins, sync=False,
                           reason="ACT table-set order (sqrt first)")
            tmp = cons.tile([TOK, 256], F32)
            nc.vector.tensor_tensor(tmp, y1, th, AX.mult)
            y1s = cons.tile([TOK, 256], F32)
            nc.vector.tensor_tensor(y1s, tmp, y1, AX.add)

            y1sT = cons.tile([128, 2, TOK], F32)
            for j in range(2):
                p_yt = ptp.tile([128, 128], F32, tag="pt")
                nc.tensor.transpose(p_yt, y1s[:, j * 128 : (j + 1) * 128],
                                    ident)
                nc.scalar.copy(y1sT[:, j, :], p_yt)

            p_z = pss.tile([TOK, K], F32, tag="ps")
            nc.tensor.matmul(p_z, lhsT=y1sT[:, 0, :], rhs=w2m[:, 0, :],
                             start=True, stop=False)
            nc.tensor.matmul(p_z, lhsT=y1sT[:, 1, :], rhs=w2m[:, 1, :],
                             start=False, stop=not has_b2)
            if has_b2:
                nc.tensor.matmul(p_z, lhsT=ones32, rhs=b2r,
                                 start=False, stop=True)
            mx = cons.tile([TOK, 1], F32)
            nc.vector.reduce_max(mx, p_z, axis=mybir.AxisListType.X)
            nmx = cons.tile([TOK, 1], F32)
            nc.vector.tensor_scalar(nmx, mx, -1.0, None, AX.mult)
            ez = cons.tile([TOK, K], F32)
            esum = cons.tile([TOK, 1], F32)
            nc.scalar.activation(ez, p_z, AF.Exp, bias=nmx, accum_out=esum)
            rsum = cons.tile([TOK, 1], F32)
            nc.vector.reciprocal(rsum, esum)
            mixw = cons.tile([TOK, K], F32)
            nc.vector.tensor_scalar(mixw, ez, rsum, None, AX.mult)
            nc.sync.dma_start(mixw_d.ap(), mixw)

        # ---- big outputs: 2 halves x 4 banks, weights kept stationary ----
        with tc.tile_pool(name="pP", bufs=4, space="PSUM") as psP, \
             tc.tile_pool(name="pL", bufs=4, space="PSUM") as psL:
            for half in range(2):
                gs = [half * 4 + q for q in range(4)]
                Pb = {}
                for g in gs:
                    Pb[g] = psP.tile([TOK, KG, CF], F32, tag="pp", name=f"pp{g}")
                    nc.tensor.matmul(Pb[g], lhsT=aT, rhs=w2g4,
                                     start=True, stop=False)
                for g in gs:
                    sl = slice(SM_V + g * KG * CF, SM_V + (g + 1) * KG * CF)
                    nc.tensor.matmul(Pb[g], lhsT=ones16, rhs=sm[:, sl],
                                     start=False, stop=True)
                for g in gs:
                    st = stg.tile([TOK, KG, CF], F32, tag="st")
                    nc.vector.tensor_tensor(
                        st, Pb[g],
                        rstd[:, g * KG : (g + 1) * KG, None].to_broadcast(
                            (TOK, KG, CF)),
                        AX.mult)
                    if has_bb:
                        nc.vector.tensor_tensor(
                            st, st,
                            bbr[:, None, :].to_broadcast((TOK, KG, CF)),
                            AX.add)
                    mus_sb = stg.tile([TOK, KG, CF], F32, tag="mu")
                    nc.scalar.activation(mus_sb, st, AF.Tanh)
                    nc.sync.dma_start(mus_d.ap()[:, g * KG : (g + 1) * KG, :],
                                      mus_sb)
                Lb = {}
                for g in gs:
                    Lb[g] = psL.tile([TOK, KG, CF], F32, tag="pl", name=f"pl{g}")
                    nc.tensor.matmul(Lb[g], lhsT=alvT, rhs=i4r,
                                     start=True, stop=False)
                for g in gs:
                    sl = slice(SM_C + g * KG * CF, SM_C + (g + 1) * KG * CF)
                    nc.tensor.matmul(Lb[g], lhsT=ones16, rhs=sm[:, sl],
                                     start=False, stop=True)
                for g in gs:
                    lv_sb = stg.tile([TOK, KG, CF], F32, tag="lv")
                    nc.vector.tensor_scalar(lv_sb, Lb[g], LV_MAX, LV_MIN,
                                            AX.min, AX.max)
                    nc.sync.dma_start(lv_d.ap()[:, g * KG : (g + 1) * KG, :],
                                      lv_sb)

    if split_waits:
        _split_drain_waits(nc)
    return nc


def prepare(inputs):
    """Host-side preprocessing -> (in_maps, flags). All heavy per-token work
    stays on device; only (K,CD)-sized code/weight constants are folded."""
    f64 = {k: np.asarray(v, np.float64) for k, v in inputs.items()}
    h = np.ascontiguousarray(np.asarray(inputs["h"], np.float32))

    cm = MOM * f64["code_momentum"] + (1.0 - MOM) * f64["code_embed"]
    Cmu = cm @ f64["mu_w1"][D:] + f64["mu_b1"]          # (K, CF)
    mC = Cmu.mean(-1, keepdims=True)
    Chat = Cmu - mC
    vC = (Chat**2).mean(-1)                              # (K,)
    W2g = f64["ln_g"][:, None] * f64["mu_w2"]            # (CF, CF)
    V = Chat @ W2g                                       # (K, CF)
    bbias = f64["ln_b"] @ f64["mu_w2"] + f64["mu_b2"]    # (CF,)
    Clv = cm @ f64["lv_w"][D:] + f64["lv_b"]             # (K, CF)

    import ml_dtypes
    c = lambda a: np.ascontiguousarray(np.asarray(a, np.float32))
    w1s = c(f64["mw_w1"].reshape(DCH, 128, 256).transpose(1, 0, 2))
    wmu = f64["mu_w1"][:D].reshape(DCH, 128, CF).transpose(1, 0, 2)
    wlv = f64["lv_w"][:D].reshape(DCH, 128, CF).transpose(1, 0, 2)
    wml = c(np.concatenate([wmu, wlv], axis=2))          # (128, DCH, 256)
    w2m = c((0.5 * f64["mw_w2"]).reshape(2, 128, K).transpose(1, 0, 2))
    w2g4 = c(np.tile(W2g, (1, KG)))
    chT = c(Chat.T)
    pack32 = np.zeros((128, P32_LEN), np.float32)
    # h filled per core below
    pack32[:, ID:WML] = np.eye(128, dtype=np.float32)
    pack32[:, WML:W1S] = wml.reshape(128, -1)
    pack32[:, W1S:W2G] = w1s.reshape(128, -1)
    pack32[:, W2G:I4C] = w2g4
    pack32[:, I4C:W2M] = np.tile(np.eye(128), (1, KG))
    pack32[:, W2M:CHT] = w2m.reshape(128, -1)
    pack32[:, CHT:ON32] = chT
    pack32[0, ON32:VCS] = 1.0
    pack32[0, VCS:P32_LEN] = (CF / 2.0) * vC
    smalls = np.zeros((1, SM_LEN), ml_dtypes.bfloat16)
    smalls[0, SM_V : SM_V + K * CF] = V.reshape(-1).astype(ml_dtypes.bfloat16)
    smalls[0, SM_C : SM_C + K * CF] = Clv.reshape(-1).astype(
        ml_dtypes.bfloat16)
    smalls[0, SM_1 : SM_1 + 128] = 1.0

    has_b1 = bool(np.any(f64["mw_b1"]))
    has_b2 = bool(np.any(f64["mw_b2"]))
    has_bb = bool(np.any(bbias))

    common = dict(smalls16=smalls)
    if has_b1:
        common["b1row"] = c(f64["mw_b1"].reshape(1, 256))
    if has_b2:
        common["b2row"] = c(f64["mw_b2"].reshape(1, K))
    if has_bb:
        common["bbrep"] = c(np.tile(bbias.reshape(1, CF), (128, 1)))

    in_maps = []
    for i in range(NCORES):
        m = dict(common)
        p = pack32.copy()
        p[:, H0:ID] = h[i * BPC : (i + 1) * BPC].reshape(TOK, D)
        m["pack32"] = p
        in_maps.append(m)
    return in_maps, (has_b1, has_b2, has_bb)


_CACHE = {}


def run(inputs, **spmd_kwargs):
    in_maps, flags = prepare(inputs)
    if flags not in _CACHE:
        _CACHE[flags] = build_bass(*flags)
    nc = _CACHE[flags]
    res = run_bass_kernel_spmd(nc, in_maps, core_ids=list(range(NCORES)),
                               **spmd_kwargs)
    mix = np.empty((B, T, K), np.float32)
    mus = np.empty((B, T, K, CF), np.float32)
    lv = np.empty((B, T, K, CF), np.float32)
    for i, r in enumerate(res.results):
        sl = slice(i * BPC, (i + 1) * BPC)
        mix[sl] = r["mixw"].reshape(BPC, T, K)
        mus[sl] = r["mus"].reshape(BPC, T, K, CF)
        lv[sl] = r["lv"].reshape(BPC, T, K, CF)
    return (mix, mus, lv), res


def kernel(**inputs):
    out, _ = run(inputs)
    return out
